# revision 1
# baseline (speedup 1.0000x reference)
"""Trainium2 Bass kernel: LocalCausalTransformerBlock (window-3 causal attention).

Sharding: 8-way sequence-parallel. B=2 x N=2048 = 4096 tokens -> 8 chunks of
512 tokens (4 chunks per batch row). Each core gets its 512 tokens plus a
2-token halo (the preceding tokens of the same sequence) so the window-3
causal attention needs no cross-core communication. Weights are replicated.

The four big matmuls (qkv/proj/fc1/fc2) run in fp8e4m3 with DoubleRow perf
mode (contract 2x128 channels per instruction at 0.5 cycles/row). Weights are
pre-scaled per output column to a power of two near absmax~2 so e4m3's
subnormal range is never hit; the descale rides the eviction's free
multiplicative scalar slot. qkv and proj additionally carry the quantization
residual ("lo") as extra fp8 k-chunks in the same accumulation group.
Attention internals (q/k/v, softmax, AV) are bf16; LayerNorm stats, softmax
normalizer and both residual streams are fp32.

Layout: activations live "transposed" (channels on partitions, tokens on the
free axis) so matmuls contract over partitions and the +-1/+-2 token shifts
of the local attention are free-axis offsets. LayerNorm runs token-major; PE
transposes bridge the layouts, batched 8-to-a-psum-bank with one wide strided
eviction. Softmax needs no max-subtraction (window-3 scores are small): exp
runs directly on the score PSUM; a per-core multiplicative edge mask zeroes
out-of-window columns after exp. Act-table funcs are ordered
sqrt->exp->sqrt->gelu (4 loads; identity is in every set). Weight matrices
stream in as column-block DMAs so matmuls start before the full matrix
lands; small constants ride in two packed DMAs. PSUM evictions are spread
across DVE and Act (gpsimd cannot touch PSUM); Pool takes SBUF-only work.
"""

import sys

for _p in ("/opt/trn_rl_repo",):
    if _p not in sys.path:
        sys.path.insert(0, _p)

import numpy as np
import ml_dtypes

P = 128
D = 1024
H = 16
HD = 64
H3 = 3 * D
HID = 4096
T = 512            # real tokens per core
TH = T + 2         # k/v token axis with 2-token halo (halo stored first)
NCORE = 8
EPS = 1e-5
BF = ml_dtypes.bfloat16
F8 = ml_dtypes.float8_e4m3

# which weights carry the fp8 quantization residual (2x k-chunks)
COMP = {"qkv": False, "proj": False, "fc1": False, "fc2": False}

# packed f32 const columns
_C = {}
_off = 0
for _name, _w in [("qkvb", 24), ("qkvs", 24), ("projb", 8), ("projs", 8),
                  ("fc1b", 32), ("fc1s", 32), ("fc2b", 8), ("fc2s", 8),
                  ("khs", 32), ("khb", 32)]:
    _C[_name] = _off
    _off += _w
CPAK_W = _off
# packed bf16 const columns: idb, hmask, emk, emask
_B = {"idb": 0, "hmask": 128, "emk": 256, "emask": 259}
BPAK_W = 259 + 1024

_CACHE: dict = {}


def _build_program():
    import concourse.bass as bass
    import concourse.tile as tile
    from concourse import bacc, mybir
    from contextlib import ExitStack

    f32 = mybir.dt.float32
    bf16 = mybir.dt.bfloat16
    fp8 = mybir.dt.float8e4
    ALU = mybir.AluOpType
    ACT = mybir.ActivationFunctionType
    DR = mybir.MatmulPerfMode.DoubleRow

    KQ = 16 if COMP["qkv"] else 8
    KP = 16 if COMP["proj"] else 8
    K1 = 16 if COMP["fc1"] else 8
    K2 = 64 if COMP["fc2"] else 32

    nc = bacc.Bacc()

    xh_d = nc.declare_dram_parameter("xh", [2, D], bf16, isOutput=False)
    xm_d = nc.declare_dram_parameter("xm", [T, D], bf16, isOutput=False)
    qkvw_ds = [nc.declare_dram_parameter(f"qkvw{b}", [P, KQ * 768], fp8,
                                         isOutput=False) for b in range(4)]
    projw_d = nc.declare_dram_parameter("projw", [P, KP * D], fp8, isOutput=False)
    fc1w_ds = [nc.declare_dram_parameter(f"fc1w{b}", [P, K1 * 2048], fp8,
                                         isOutput=False) for b in range(2)]
    fc2w_ds = [nc.declare_dram_parameter(f"fc2w{b}", [P, K2 * 512], fp8,
                                         isOutput=False) for b in range(2)]
    cpak_d = nc.declare_dram_parameter("cpak", [P, CPAK_W], f32, isOutput=False)
    bpak_d = nc.declare_dram_parameter("bpak", [P, BPAK_W], bf16, isOutput=False)
    out_d = nc.declare_dram_parameter("out", [T, D], bf16, isOutput=True)

    with tile.TileContext(nc) as tc, ExitStack() as ctx:
        # PSUM budget (8 banks): mm x4, sc x2 (scores <-> fc2), tp x2
        const = ctx.enter_context(tc.tile_pool(name="const", bufs=1))
        acts = ctx.enter_context(tc.tile_pool(name="acts", bufs=1))
        ln_pool = ctx.enter_context(tc.tile_pool(name="ln", bufs=2))
        tp_ps = ctx.enter_context(tc.tile_pool(name="tp_ps", bufs=2, space="PSUM"))
        mm_ps = ctx.enter_context(tc.tile_pool(name="mm_ps", bufs=4, space="PSUM"))
        sc_ps = ctx.enter_context(tc.tile_pool(name="sc_ps", bufs=2, space="PSUM"))

        bpak = const.tile([P, BPAK_W], bf16, tag="bp", name="bpak")
        nc.sync.dma_start(bpak[:], bpak_d[:])
        cpak = const.tile([P, CPAK_W], f32, tag="cp", name="cpak")

        def cp(name, j, w=1):
            o = _C[name] + j
            return cpak[:, o:o + w]

        idb = bpak[:, _B["idb"]:_B["idb"] + 128]
        hmask = bpak[:, _B["hmask"]:_B["hmask"] + 128]
        emk = bpak[0:H, _B["emk"]:_B["emk"] + 3]
        emask = bpak[0:H, _B["emask"]:_B["emask"] + 1024]

        # activations alive into the MLP phases
        x2t = acts.tile([P, 4 * D], bf16, tag="x2t", name="x2t")
        x2lnT = acts.tile([P, 8, T], fp8, tag="x2lnT", name="x2lnT")
        hT = acts.tile([P, 32, T], fp8, tag="hT", name="hT")
        # weights preloaded early so their DMAs overlap earlier phases
        projw = acts.tile([P, KP, D], fp8, tag="projw", name="projw")
        fc1w = [acts.tile([P, K1, 2048], fp8, tag=f"fc1w{b}", name=f"fc1w{b}")
                for b in range(2)]

        def layernorm_tok(src_ap, s, dstT, dst_off, pool=None, tp_tag="tp",
                          tp_pool=None, split_apply=False, evict="act"):
            """Token-major LN over s tokens -> fp8 channel-major in
            dstT[:, ch, dst_off:dst_off+s]. Stats on DVE, rstd via Act sqrt +
            DVE reciprocal, apply on Pool (optionally split Pool/DVE),
            transpose batch on PE, one wide Act eviction."""
            pool = pool or ln_pool
            tp_pool = tp_pool or tp_ps
            stat = pool.tile([s, 12], bf16, tag=f"lnstat{s}", name=f"st{s}")
            nc.vector.bn_stats(stat[:, 0:6], src_ap[:, 0:512])
            nc.vector.bn_stats(stat[:, 6:12], src_ap[:, 512:1024])
            mv = pool.tile([s, 2], f32, tag=f"lnmv{s}", name=f"mv{s}")
            nc.vector.bn_aggr(mv[:], stat[:])
            vpe = pool.tile([s, 1], f32, tag=f"lnvpe{s}", name=f"vpe{s}")
            nc.vector.tensor_scalar_add(vpe[:], mv[:, 1:2], EPS)
            std = pool.tile([s, 1], f32, tag=f"lnstd{s}", name=f"sd{s}")
            nc.scalar.activation(std[:], vpe[:], ACT.Sqrt)
            rstd = pool.tile([s, 1], f32, tag=f"lnrstd{s}", name=f"rs{s}")
            nc.vector.reciprocal(rstd[:], std[:])
            nmr = pool.tile([s, 1], f32, tag=f"lnnmr{s}", name=f"nm{s}")
            nc.vector.scalar_tensor_tensor(
                nmr[:], mv[:, 0:1], -1.0, rstd[:], ALU.mult, ALU.mult
            )
            xln = pool.tile([s, D], bf16, tag=f"lnout{s}", name=f"xo{s}")
            if split_apply:
                nc.gpsimd.tensor_scalar(xln[:, 0:512], src_ap[:, 0:512],
                                        rstd[:, 0:1], nmr[:, 0:1],
                                        ALU.mult, ALU.add)
                nc.vector.tensor_scalar(xln[:, 512:1024], src_ap[:, 512:1024],
                                        rstd[:, 0:1], nmr[:, 0:1],
                                        ALU.mult, ALU.add)
            else:
                nc.gpsimd.tensor_scalar(xln[:], src_ap[:], rstd[:, 0:1],
                                        nmr[:, 0:1], ALU.mult, ALU.add)
            tpw = tp_pool.tile([P, 8, s], bf16, tag=tp_tag, name=f"tpln{s}")
            for ch in range(8):
                nc.tensor.transpose(tpw[:, ch, :], xln[:, ch * P:(ch + 1) * P],
                                    idb[0:s, 0:s])
            if evict == "act":
                nc.scalar.activation(dstT[:, :, dst_off:dst_off + s], tpw[:],
                                     ACT.Identity)
            else:
                nc.vector.tensor_copy(dstT[:, :, dst_off:dst_off + s], tpw[:])

        with tc.tile_pool(name="p1", bufs=1) as p1:
            xt = p1.tile([P, 4 * D], bf16, tag="xt", name="xt")
            xh = p1.tile([2, D], bf16, tag="xh", name="xh")
            xlnT = p1.tile([P, 8, T], fp8, tag="xlnT", name="xlnT")
            xlnTh = p1.tile([P, 8, 2], fp8, tag="xlnTh", name="xlnTh")
            qT = p1.tile([P, 8 * T], bf16, tag="qT", name="qT")
            kvT = p1.tile([P, 16, TH], bf16, tag="kvT", name="kvT")

            nc.sync.dma_start(xh[:], xh_d[:])
            for ti in range(4):
                for hf in range(2):
                    nc.sync.dma_start(
                        xt[:, ti * D + hf * 512:ti * D + (hf + 1) * 512],
                        xm_d[ti * P:(ti + 1) * P, hf * 512:(hf + 1) * 512])
            nc.sync.dma_start(cpak[:], cpak_d[:])

            with tc.tile_pool(name="p3", bufs=1) as p3:
                attnT = p3.tile([P, 8, T], fp8, tag="attnT", name="attnT")
                with tc.tile_pool(name="p2", bufs=1) as p2:
                    et = p2.tile([H, 3, T], bf16, tag="et", name="et")
                    with tc.tile_pool(name="wq", bufs=1) as wq_pool:
                        qkvw = []
                        for b in range(4):
                            t = wq_pool.tile([P, KQ, 768], fp8,
                                             tag=f"qkvw{b}", name=f"qkvw{b}")
                            nc.sync.dma_start(t[:], qkvw_ds[b][:])
                            qkvw.append(t)
                        for b in range(2):
                            nc.sync.dma_start(fc1w[b][:], fc1w_ds[b][:])
                        nc.sync.dma_start(projw[:], projw_d[:])

                        # ---- LN1 (halo first: xh lands first) ----
                        layernorm_tok(xh[:], 2, xlnTh, 0)
                        for ti in range(4):
                            layernorm_tok(xt[:, ti * D:(ti + 1) * D], P,
                                          xlnT, ti * P)

                        # ---- QKV ----
                        # halo k/v columns: one psum tile = 16 blocks x 2 cols
                        ph = tp_ps.tile([P, 8, 4], f32, tag="tp", name="ph")
                        for j in range(16):
                            col = D + j * P
                            wt = qkvw[col // 768]
                            wo = col % 768
                            for i in range(KQ // 2):
                                xc = (2 * i) % 8
                                nc.tensor.matmul(
                                    ph[:, j // 2, (j % 2) * 2:(j % 2) * 2 + 2],
                                    wt[:, 2 * i:2 * i + 2, wo:wo + P],
                                    xlnTh[:, xc:xc + 2, :],
                                    start=(i == 0), stop=(i == KQ // 2 - 1),
                                    perf_mode=DR,
                                )
                        pht = ln_pool.tile([P, 32], f32, tag="pht", name="pht")
                        nc.vector.tensor_mul(pht[:], ph[:, :, :],
                                             cp("khs", 0, 32))
                        for j in range(16):
                            nc.gpsimd.tensor_add(
                                kvT[:, j, 0:2], pht[:, 2 * j:2 * j + 2],
                                cp("khb", 2 * j, 2))

                        def qkv_tile(j):
                            wt = qkvw[j // 6]
                            wo = (j % 6) * P
                            ps = mm_ps.tile([P, T], f32, tag="mm",
                                            name=f"qkv{j}")
                            for i in range(KQ // 2):
                                xc = (2 * i) % 8
                                nc.tensor.matmul(
                                    ps[:], wt[:, 2 * i:2 * i + 2, wo:wo + P],
                                    xlnT[:, xc:xc + 2, :],
                                    start=(i == 0), stop=(i == KQ // 2 - 1),
                                    perf_mode=DR,
                                )
                            if j < 8:
                                dst = qT[:, j * T:(j + 1) * T]
                            else:
                                dst = kvT[:, j - 8, 2:TH]
                            if j % 2 == 0:
                                nc.vector.tensor_scalar(
                                    dst, ps[:], cp("qkvs", j), cp("qkvb", j),
                                    ALU.mult, ALU.add)
                            else:
                                nc.scalar.activation(dst, ps[:], ACT.Identity,
                                                     bias=cp("qkvb", j),
                                                     scale=cp("qkvs", j))

                        for j in range(16):      # q then k
                            qkv_tile(j)
                        # scores overlap the v-tile matmuls below
                        for w in range(3):
                            e = p2.tile([P, 4, T], bf16, tag="e", bufs=2,
                                        name=f"e{w}")
                            e2 = p2.tile([P, 4, T], bf16, tag="e", bufs=2,
                                         name=f"e2{w}")
                            nc.vector.tensor_mul(
                                e[:], qT[:, 0:4 * T],
                                kvT[:, 0:4, 2 - w:2 - w + T])
                            nc.vector.tensor_mul(
                                e2[:], qT[:, 4 * T:8 * T],
                                kvT[:, 4:8, 2 - w:2 - w + T])
                            sc = sc_ps.tile([H, T], f32, tag="sc",
                                            name=f"sc{w}")
                            for ch in range(8):
                                esrc = e if ch < 4 else e2
                                nc.tensor.matmul(
                                    sc[:], hmask[:, ch * H:(ch + 1) * H],
                                    esrc[:, ch % 4, :],
                                    start=(ch == 0), stop=(ch == 7),
                                )
                            nc.scalar.activation(et[:, w, :], sc[:], ACT.Exp)
                        # preload the sqrt act table for LN2 while Act
                        # has slack (identity is in every table)
                        scr = ln_pool.tile([P, 1], f32, tag="scr", name="scr")
                        nc.scalar.activation(scr[:], cp("qkvs", 0), ACT.Sqrt)
                        # ---- softmax (before the v evictions so pw is
                        # ready when the PE reaches the bc matmuls) ----
                        nc.gpsimd.tensor_mul(et[:, 1, 0:1], et[:, 1, 0:1],
                                             emk[:, 0:1])
                        nc.gpsimd.tensor_mul(et[:, 2, 0:2], et[:, 2, 0:2],
                                             emk[:, 1:3])
                        z0 = p2.tile([H, T], bf16, tag="z0", name="z0")
                        z1 = p2.tile([H, T], bf16, tag="z1", name="z1")
                        rz = p2.tile([H, T], bf16, tag="rz", name="rz")
                        nc.gpsimd.tensor_add(z0[:], et[:, 0, :], et[:, 1, :])
                        nc.gpsimd.tensor_add(z1[:], z0[:], et[:, 2, :])
                        with nc.allow_low_precision(reason="softmax bf16"):
                            nc.vector.reciprocal(rz[:], z1[:])
                        pw = p2.tile([H, 3, T], bf16, tag="pw", name="pw")
                        for w in range(3):
                            nc.vector.tensor_mul(pw[:, w, :], et[:, w, :],
                                                 rz[:])
                        for j in range(16, 24):  # v
                            qkv_tile(j)

                        # broadcast probs to channels (psum -> SBUF bf16 so
                        # the AV muls run 2x); overlaps the v evictions
                        bcs = p2.tile([P, 8, 3, T], bf16, tag="bcs",
                                      name="bcs")
                        for chp in range(4):  # chunk pairs, fully streamed
                            ch = 2 * chp
                            for c in (ch, ch + 1):
                                for w in range(3):
                                    bc = mm_ps.tile([P, T], f32, tag="mm",
                                                    name=f"bc{c}_{w}")
                                    nc.tensor.matmul(
                                        bc[:], emask[:, c * P:(c + 1) * P],
                                        pw[:, w, :], start=True, stop=True,
                                    )
                                    if c % 4 == 0:
                                        nc.vector.tensor_copy(
                                            bcs[:, c, w, :], bc[:])
                                    else:
                                        nc.scalar.activation(
                                            bcs[:, c, w, :], bc[:],
                                            ACT.Identity)
                            avs = []
                            for w in range(3):
                                av = p2.tile([P, 2, T], bf16, tag="av",
                                             bufs=4, name=f"av{chp}_{w}")
                                nc.vector.tensor_mul(
                                    av[:], bcs[:, ch:ch + 2, w, :],
                                    kvT[:, 8 + ch:10 + ch, 2 - w:2 - w + T],
                                )
                                avs.append(av)
                            av01 = p2.tile([P, 2, T], bf16, tag="av01",
                                           bufs=2, name=f"av01_{chp}")
                            eng = nc.vector if chp == 3 else nc.gpsimd
                            eng.tensor_add(av01[:], avs[0][:], avs[1][:])
                            eng.tensor_add(attnT[:, ch:ch + 2, :],
                                           av01[:], avs[2][:])

                # ---- proj + residual 1 + LN2 ----
                with tc.tile_pool(name="p5", bufs=1) as p5:
                    ln2_pool = p5  # p5-scoped temps, deeper rotation
                    yT = p5.tile([P, 8 * T], bf16, tag="yT", name="yT")
                    # 8 concurrent psum groups streaming over attnT pairs
                    pjps = {}
                    for j in range(8):
                        pool, tag = [(sc_ps, "sc"), (mm_ps, "mm"),
                                     (tp_ps, "tp")][0 if j < 2 else
                                                    (1 if j < 6 else 2)]
                        pjps[j] = pool.tile([P, T], f32, tag=tag,
                                            name=f"pj{j}")
                    for i in range(KP // 2):
                        for j in range(8):
                            nc.tensor.matmul(
                                pjps[j][:], projw[:, 2 * i:2 * i + 2,
                                                  j * P:(j + 1) * P],
                                attnT[:, 2 * i:2 * i + 2, :],
                                start=(i == 0), stop=(i == KP // 2 - 1),
                                perf_mode=DR,
                            )
                    for j in range(8):
                        if j % 2 == 0:
                            nc.vector.tensor_scalar(yT[:, j * T:(j + 1) * T],
                                                    pjps[j][:],
                                                    cp("projs", j),
                                                    cp("projb", j), ALU.mult,
                                                    ALU.add)
                        else:
                            nc.scalar.activation(yT[:, j * T:(j + 1) * T],
                                                 pjps[j][:], ACT.Identity,
                                                 bias=cp("projb", j),
                                                 scale=cp("projs", j))
                    with tc.tile_pool(name="ln2", bufs=4) as ln2p:
                      for ti in range(4):
                        for g in range(2):
                            tpw = tp_ps.tile([P, 4, P], bf16, tag="tp",
                                             name=f"tpy{ti}_{g}")
                            for ch in range(4):
                                nc.tensor.transpose(
                                    tpw[:, ch, :],
                                    yT[:, (4 * g + ch) * T + ti * P:
                                       (4 * g + ch) * T + (ti + 1) * P],
                                    idb[:, :])
                            c0 = ti * D + g * 4 * P
                            nc.vector.tensor_add(
                                x2t[:, c0:c0 + 4 * P],
                                xt[:, c0:c0 + 4 * P], tpw[:])
                        layernorm_tok(x2t[:, ti * D:(ti + 1) * D], P,
                                      x2lnT, ti * P, pool=ln2p,
                                      tp_tag="sc", tp_pool=sc_ps,
                                      split_apply=True,
                                      evict="dve" if ti == 3 else "act")
                    scr2 = ln_pool.tile([P, 1], f32, tag="scr", name="scr2")
                    nc.scalar.activation(scr2[:], cp("qkvs", 0), ACT.Gelu)

        # ---- MLP fc1 + gelu, fc2 + residual 2 + store ----
        with tc.tile_pool(name="w1", bufs=1) as w1_pool:
            with tc.tile_pool(name="w2", bufs=1) as w2_pool:
                fc2w = []
                for b in range(2):
                    t = w2_pool.tile([P, K2, 512], fp8, tag=f"fc2w{b}",
                                     name=f"fc2w{b}")
                    nc.sync.dma_start(t[:], fc2w_ds[b][:])
                    fc2w.append(t)
                outt = w2_pool.tile([P, 4 * D], bf16, tag="outt", name="outt")
                mT = w2_pool.tile([P, 8 * T], bf16, tag="mT", name="mT")

                for j in range(32):
                    wt = fc1w[j // 16]
                    wo = (j % 16) * P
                    ps = mm_ps.tile([P, T], f32, tag="mm", name=f"f1{j}")
                    for i in range(K1 // 2):
                        xc = (2 * i) % 8
                        nc.tensor.matmul(
                            ps[:], wt[:, 2 * i:2 * i + 2, wo:wo + P],
                            x2lnT[:, xc:xc + 2, :],
                            start=(i == 0), stop=(i == K1 // 2 - 1),
                            perf_mode=DR,
                        )
                    nc.scalar.activation(hT[:, j, :], ps[:], ACT.Gelu,
                                         bias=cp("fc1b", j),
                                         scale=cp("fc1s", j))

                # fc2: 6 concurrent psum groups, pair-major so matmuls ride
                # the gelu eviction cadence; last 2 groups after
                def f2_mm(ps, j, i):
                    wt = fc2w[j // 4]
                    wo = (j % 4) * P
                    xc = (2 * i) % 32
                    nc.tensor.matmul(
                        ps[:], wt[:, 2 * i:2 * i + 2, wo:wo + P],
                        hT[:, xc:xc + 2, :],
                        start=(i == 0), stop=(i == K2 // 2 - 1),
                        perf_mode=DR,
                    )

                def f2_evict(ps, j):
                    if j % 2 == 0:
                        nc.vector.tensor_scalar(mT[:, j * T:(j + 1) * T],
                                                ps[:], cp("fc2s", j),
                                                cp("fc2b", j), ALU.mult,
                                                ALU.add)
                    else:
                        nc.scalar.activation(mT[:, j * T:(j + 1) * T], ps[:],
                                             ACT.Identity, bias=cp("fc2b", j),
                                             scale=cp("fc2s", j))

                f2ps = {}
                for j in range(6):
                    pool = sc_ps if j < 2 else mm_ps
                    f2ps[j] = pool.tile([P, T], f32,
                                        tag="sc" if j < 2 else "mm",
                                        name=f"f2{j}")
                for i in range(K2 // 2):
                    for j in range(6):
                        f2_mm(f2ps[j][:], j, i)
                for j in range(6):
                    f2_evict(f2ps[j][:], j)
                for j in (6, 7):
                    ps = sc_ps.tile([P, T], f32, tag="sc", name=f"f2{j}")
                    for i in range(K2 // 2):
                        f2_mm(ps[:], j, i)
                    f2_evict(ps[:], j)

                for ti in range(4):
                    for g in range(2):
                        tpw = tp_ps.tile([P, 4, P], bf16, tag="tp",
                                         name=f"tpm{ti}_{g}")
                        for ch in range(4):
                            nc.tensor.transpose(
                                tpw[:, ch, :],
                                mT[:, (4 * g + ch) * T + ti * P:
                                   (4 * g + ch) * T + (ti + 1) * P],
                                idb[:, :])
                        c0 = ti * D + g * 4 * P
                        nc.vector.tensor_add(
                            outt[:, c0:c0 + 4 * P],
                            x2t[:, c0:c0 + 4 * P], tpw[:])
                        nc.sync.dma_start(
                            out_d[ti * P:(ti + 1) * P, g * 512:(g + 1) * 512],
                            outt[:, c0:c0 + 4 * P])

    if not nc.is_finalized():
        nc.finalize()
    return nc


def _scale_w(w):
    amax = np.abs(w).max(axis=0, keepdims=True)
    s = 2.0 ** np.round(np.log2(2.0 / np.maximum(amax, 1e-30)))
    return w * s, (1.0 / s)[0]


def _prep_w(w, comp):
    """[Din, Dout] fp32 -> ([128, kchunks, Dout] fp8 chunk-major hi(+lo),
    descale vector [Dout])."""
    din, dout = w.shape
    nch = din // P
    ws, descale = _scale_w(np.ascontiguousarray(w.astype(np.float32)))
    hi = ws.astype(F8)
    blocks = [hi]
    if comp:
        lo = (ws - hi.astype(np.float32)).astype(F8)
        blocks.append(lo)
    cols = []
    for b in blocks:
        cols.append(b.reshape(nch, P, dout).transpose(1, 0, 2))
    out = np.concatenate(cols, axis=1)  # [128, kchunks, dout]
    return np.ascontiguousarray(out), descale.astype(np.float32)


def _host_inputs(x, qkv_w, qkv_b, proj_w, proj_b, g1, b1, g2, b2,
                 fc1_w, fc1_b, fc2_w, fc2_b):
    scale = HD ** -0.5
    qkvw_eff = (qkv_w * g1[:, None]).astype(np.float32).copy()
    qkvb_eff = (qkv_b + b1 @ qkv_w).astype(np.float32).copy()
    qkvw_eff[:, 0:D] *= scale
    qkvb_eff[0:D] *= scale
    fc1w_eff = (fc1_w * g2[:, None]).astype(np.float32)
    fc1b_eff = (fc1_b + b2 @ fc1_w).astype(np.float32)

    qkvw_p, qkvs_v = _prep_w(qkvw_eff, COMP["qkv"])
    projw_p, projs_v = _prep_w(proj_w.astype(np.float32), COMP["proj"])
    fc1w_p, fc1s_v = _prep_w(fc1w_eff, COMP["fc1"])
    fc2w_p, fc2s_v = _prep_w(fc2_w.astype(np.float32), COMP["fc2"])

    cpak = np.zeros((P, CPAK_W), np.float32)

    def setc(name, vec, n):
        cpak[:, _C[name]:_C[name] + n] = vec.reshape(n, P).T

    setc("qkvb", qkvb_eff, 24)
    setc("qkvs", qkvs_v, 24)
    setc("projb", proj_b.astype(np.float32), 8)
    setc("projs", projs_v, 8)
    setc("fc1b", fc1b_eff, 32)
    setc("fc1s", fc1s_v, 32)
    setc("fc2b", fc2_b.astype(np.float32), 8)
    setc("fc2s", fc2s_v, 8)
    kv_s = qkvs_v[D:3 * D].reshape(16, P)
    kv_b = qkvb_eff[D:3 * D].reshape(16, P)
    for j in range(16):
        for c in range(2):
            cpak[:, _C["khs"] + 2 * j + c] = kv_s[j]
            cpak[:, _C["khb"] + 2 * j + c] = kv_b[j]

    bpak0 = np.zeros((P, BPAK_W), np.float32)
    bpak0[:, _B["idb"]:_B["idb"] + 128] = np.eye(P)
    hm = np.zeros((P, 8, H), np.float32)
    for c in range(P):
        for ch in range(8):
            hm[c, ch, 2 * ch + c // HD] = 1.0
    bpak0[:, _B["hmask"]:_B["hmask"] + 128] = hm.reshape(P, 8 * H)
    em = np.zeros((H, 8, P), np.float32)
    for ch in range(8):
        for m in range(P):
            em[2 * ch + m // HD, ch, m] = 1.0
    bpak0[0:H, _B["emask"]:_B["emask"] + 1024] = em.reshape(H, 8 * P)

    common = {
        "projw": np.ascontiguousarray(projw_p.reshape(P, -1)),
        "cpak": cpak,
    }
    for b in range(4):
        common[f"qkvw{b}"] = np.ascontiguousarray(
            qkvw_p[:, :, b * 768:(b + 1) * 768].reshape(P, -1))
    for b in range(2):
        common[f"fc1w{b}"] = np.ascontiguousarray(
            fc1w_p[:, :, b * 2048:(b + 1) * 2048].reshape(P, -1))
    for b in range(2):
        common[f"fc2w{b}"] = np.ascontiguousarray(
            fc2w_p[:, :, b * 512:(b + 1) * 512].reshape(P, -1))

    in_maps = []
    for core in range(NCORE):
        b, q = divmod(core, 4)
        xm = np.ascontiguousarray(x[b, q * T:(q + 1) * T, :]).astype(BF)
        bpak = bpak0.copy()
        if q == 0:
            xhv = np.zeros((2, D), BF)
            # emk stays zero
        else:
            xhv = np.ascontiguousarray(x[b, q * T - 2:q * T, :]).astype(BF)
            bpak[0:H, _B["emk"]:_B["emk"] + 3] = 1.0
        m = dict(common)
        m["xm"] = xm
        m["xh"] = xhv
        m["bpak"] = bpak.astype(BF)
        in_maps.append(m)
    return in_maps


def kernel(**inputs) -> np.ndarray:
    from concourse.bass_utils import run_bass_kernel_spmd

    if "nc" not in _CACHE:
        _CACHE["nc"] = _build_program()
    nc = _CACHE["nc"]
    in_maps = _host_inputs(**inputs)
    res = run_bass_kernel_spmd(nc, in_maps, list(range(NCORE)))
    outs = res.results
    full = np.zeros((2, 2048, D), np.float32)
    for core in range(NCORE):
        b, q = divmod(core, 4)
        full[b, q * T:(q + 1) * T, :] = outs[core]["out"].astype(np.float32)
    return full

